# revision 11
# baseline (speedup 1.0000x reference)
"""Trainium2 Bass kernel for nn_EnhancedDownSample (conv+BN+SiLU -> 3 Mamba blocks -> LN).

Self-contained: hardcodes all shapes. Shards batch (8) across 8 NeuronCores,
with a tiny AllReduce for the BatchNorm batch statistics.

Layout summary (per core, batch=1):
  - conv front-end: 5-tap stride-2 matmuls into PSUM, BN stats via ACT accum,
    cross-core AllReduce of (sum, sumsq), BN+SiLU fused in one ACT op,
    PE transposes to build the (t-rows, 256) residual buffer.
  - per Mamba block: LN (bn_stats) -> PE transposes -> in_proj GEMM ->
    depthwise conv as diagonal matmuls -> xproj/dtproj GEMMs -> softplus ->
    selective scan via tensor_tensor_scan with rows=d_inner, free=(s-major, t),
    a-build on ACT (exp with per-partition scale A[:,s]), b = dtu*B_rep and
    y = sum_s h_s*C_rep_s on DVE, B/C broadcast by DMA partition-stride-0.
  - final LN + PE transpose back to (256, L).
"""
import json
import os
import sys

sys.path.insert(0, "/opt/trn_rl_repo")

import numpy as np
import ml_dtypes

# ---------------------------------------------------------------- constants
B_, CIN, L_IN = 8, 128, 8192
COUT, K, STRIDE = 256, 5, 2
NBLK = 3
D_INNER, D_STATE, D_CONV, DT_RANK = 512, 16, 4, 16
EPS = 1e-5
L = L_IN // STRIDE            # 4096
DG = D_INNER // 128           # 4 d-groups
T = 512                       # chunk length along t
NCHUNK = L // T               # 8
SH = 2                        # s-halves (8 states each)
SHS = D_STATE // SH           # 8
NT = L // 128                 # 32 t-tiles
N_CORES = 8

BF16 = ml_dtypes.bfloat16

# ------------------------------------------------------------- BIR legalizer
def _legalize_bir_waits(bir_bytes: bytes) -> bytes:
    """This container's walrus rejects >1 sync wait per instruction; split
    extra waits onto single-wait NoOps inserted before the instruction."""
    d = json.loads(bir_bytes)
    ctr = 0
    changed = False
    for fn in d["functions"]:
        for blk in fn["blocks"]:
            out = []
            for inst in blk["instructions"]:
                si = inst.get("sync_info")
                ow = (si or {}).get("on_wait") or []
                if len(ow) > 1:
                    changed = True
                    for w in ow[:-1]:
                        ctr += 1
                        nop = {
                            "engine": inst["engine"],
                            "ins": [],
                            "name": f"I-waitsplit-{ctr}",
                            "opcode": "NoOp",
                            "outs": [],
                            "sync_info": {"on_update": [], "on_wait": [w]},
                        }
                        if inst.get("debug") is not None:
                            nop["debug"] = inst["debug"]
                        out.append(nop)
                    si["on_wait"] = [ow[-1]]
                out.append(inst)
            blk["instructions"] = out
    return json.dumps(d).encode() if changed else bir_bytes


_PATCHED = False


def _install_wait_legalizer():
    global _PATCHED
    if _PATCHED:
        return
    import concourse.bass2jax as bass2jax
    import concourse.bass_utils as bass_utils

    orig = bass_utils.compile_bir_kernel

    def patched(bir_json, tmpdir, neff_name="file.neff"):
        return orig(_legalize_bir_waits(bytes(bir_json)), tmpdir, neff_name)

    bass2jax.compile_bir_kernel = patched
    bass_utils.compile_bir_kernel = patched
    _PATCHED = True


# ------------------------------------------------------------- device program
def build_program(n_cores: int = N_CORES):
    import concourse.bass as bass
    import concourse.tile as tile
    import concourse.mybir as mybir

    dt = mybir.dt
    AF = mybir.ActivationFunctionType
    OP = mybir.AluOpType

    nc = bass.Bass("TRN2", num_devices=n_cores)

    # ---- dram I/O ----
    x_in = nc.dram_tensor("x", [CIN, L_IN], dt.float32, kind="ExternalInput")
    convw = nc.dram_tensor("convw", [K, CIN, COUT], dt.bfloat16, kind="ExternalInput")
    bnpar = nc.dram_tensor("bnpar", [2, 128, 3], dt.float32, kind="ExternalInput")  # cb,g,b per half
    ident_bf = nc.dram_tensor("ident_bf", [128, 128], dt.bfloat16, kind="ExternalInput")
    ident_f32 = nc.dram_tensor("ident_f32", [128, 128], dt.float32, kind="ExternalInput")
    lnf = nc.dram_tensor("lnf", [2, COUT], dt.float32, kind="ExternalInput")  # w,b rows
    winT = nc.dram_tensor("winT", [NBLK, 2, 128, 2 * D_INNER], dt.bfloat16, kind="ExternalInput")
    winb = nc.dram_tensor("winb", [NBLK, 128, 8], dt.float32, kind="ExternalInput")  # bias per m-tile col
    cdiag = nc.dram_tensor("cdiag", [NBLK, 128, DG * D_CONV * 128], dt.bfloat16, kind="ExternalInput")
    cdb = nc.dram_tensor("cdb", [NBLK, 128, DG], dt.float32, kind="ExternalInput")
    xprojT = nc.dram_tensor("xprojT", [NBLK, 128, DG * 48], dt.bfloat16, kind="ExternalInput")
    dtprojT = nc.dram_tensor("dtprojT", [NBLK, 16, D_INNER], dt.bfloat16, kind="ExternalInput")
    dtpb = nc.dram_tensor("dtpb", [NBLK, 128, DG], dt.float32, kind="ExternalInput")
    acol = nc.dram_tensor("acol", [NBLK, 128, DG * D_STATE], dt.float32, kind="ExternalInput")
    dcol = nc.dram_tensor("dcol", [NBLK, 128, DG], dt.float32, kind="ExternalInput")
    woutT = nc.dram_tensor("woutT", [NBLK, DG, 128, COUT], dt.bfloat16, kind="ExternalInput")

    out_c = nc.dram_tensor("out_c", [COUT, L], dt.float32, kind="ExternalOutput")
    res_out = nc.dram_tensor("res_out", [L, COUT], dt.float32, kind="ExternalOutput")

    with tile.TileContext(nc) as tc:
        with tc.tile_pool(name="persist", bufs=1) as persist, \
             tc.tile_pool(name="psum", bufs=1, space="PSUM") as psum, \
             tc.tile_pool(name="dram", bufs=1, space="DRAM") as dram:

            res = persist.tile([128, NT * COUT], dt.float32)       # (128, 8192) residual, t-rows
            ident_b = persist.tile([128, 128], dt.bfloat16)
            nc.sync.dma_start(out=ident_b[:], in_=ident_bf[:])
            ident_f = persist.tile([128, 128], dt.float32)
            nc.sync.dma_start(out=ident_f[:], in_=ident_f32[:])
            eps_t = persist.tile([128, 1], dt.float32)
            nc.vector.memset(eps_t[:], EPS)
            lnfw_rep = persist.tile([128, COUT], dt.float32)
            nc.sync.dma_start(out=lnfw_rep[:], in_=lnf[0:1, :].to_broadcast([128, COUT]))
            lnfb_rep = persist.tile([128, COUT], dt.float32)
            nc.sync.dma_start(out=lnfb_rep[:], in_=lnf[1:2, :].to_broadcast([128, COUT]))

            # ================= front-end: conv + BN + SiLU =================
            with tc.tile_pool(name="front", bufs=1) as front, \
                 tc.tile_pool(name="front2", bufs=2) as front2:
                x_f32 = front.tile([128, L_IN], dt.float32)
                nc.sync.dma_start(out=x_f32[:], in_=x_in[:])
                x_pad = front.tile([128, L_IN + 4], dt.bfloat16)
                nc.vector.memset(x_pad[:, 0:2], 0.0)
                nc.vector.memset(x_pad[:, L_IN + 2:L_IN + 4], 0.0)
                nc.vector.tensor_copy(out=x_pad[:, 2:L_IN + 2], in_=x_f32[:])

                convw_sb = front.tile([128, K * COUT], dt.bfloat16)
                for k in range(K):
                    nc.sync.dma_start(out=convw_sb[:, k * COUT:(k + 1) * COUT], in_=convw[k])
                bnpar_sb = front.tile([128, 6], dt.float32)
                for m in range(2):
                    nc.sync.dma_start(out=bnpar_sb[:, 3 * m:3 * m + 3], in_=bnpar[m])

                h_sb = [front.tile([128, L], dt.bfloat16, name=f"h_sb{m}") for m in range(2)]
                s1p = front.tile([128, 2 * NCHUNK], dt.float32)
                s2p = front.tile([128, 2 * NCHUNK], dt.float32)
                for m in range(2):
                    for c in range(NCHUNK):
                        ph = psum.tile([128, T], dt.float32, tag="mm512", bufs=3)
                        for k in range(K):
                            nc.tensor.matmul(
                                ph[:],
                                convw_sb[:, k * COUT + m * 128:k * COUT + m * 128 + 128],
                                x_pad[:, k + c * 2 * T: k + c * 2 * T + 2 * T:2],
                                start=(k == 0), stop=(k == K - 1))
                        nc.scalar.activation(
                            out=h_sb[m][:, c * T:(c + 1) * T], in_=ph[:],
                            func=AF.Copy, accum_out=s1p[:, m * NCHUNK + c:m * NCHUNK + c + 1])
                        junk = front2.tile([128, T], dt.bfloat16, tag="junk")
                        nc.scalar.activation(
                            out=junk[:], in_=ph[:], func=AF.Square,
                            accum_out=s2p[:, m * NCHUNK + c:m * NCHUNK + c + 1])

                stats = front.tile([128, 4], dt.float32)
                for m in range(2):
                    nc.vector.tensor_reduce(
                        out=stats[:, m:m + 1], in_=s1p[:, m * NCHUNK:(m + 1) * NCHUNK],
                        axis=mybir.AxisListType.X, op=OP.add)
                    nc.vector.tensor_reduce(
                        out=stats[:, 2 + m:3 + m], in_=s2p[:, m * NCHUNK:(m + 1) * NCHUNK],
                        axis=mybir.AxisListType.X, op=OP.add)

                if n_cores > 1:
                    cc_in = dram.tile([128, 4], dt.float32)
                    cc_out = dram.tile([128, 4], dt.float32)
                    nc.gpsimd.dma_start(out=cc_in[:], in_=stats[:])
                    nc.gpsimd.collective_compute(
                        "AllReduce", OP.add,
                        replica_groups=[list(range(n_cores))],
                        ins=[cc_in[:]], outs=[cc_out[:]])
                    gstats = front.tile([128, 4], dt.float32)
                    nc.gpsimd.dma_start(out=gstats[:], in_=cc_out[:])
                    nbl = 1.0 / (B_ * L)
                else:
                    gstats = stats
                    nbl = 1.0 / L

                # per half: mean/e2/var/rstd -> scaleA/biasA
                scaleA = front.tile([128, 2], dt.float32)
                biasA = front.tile([128, 2], dt.float32)
                tmp1 = front.tile([128, 1], dt.float32)
                for m in range(2):
                    cb = bnpar_sb[:, 3 * m + 0:3 * m + 1]
                    g = bnpar_sb[:, 3 * m + 1:3 * m + 2]
                    b = bnpar_sb[:, 3 * m + 2:3 * m + 3]
                    mean = front.tile([128, 1], dt.float32, tag=f"mean{m}")
                    # mean = s1*nbl + cb
                    nc.scalar.activation(out=mean[:], in_=gstats[:, m:m + 1],
                                         func=AF.Identity, bias=cb, scale=nbl)
                    # e2 = s2*nbl + cb*(2*mean - cb)
                    e2 = front.tile([128, 1], dt.float32, tag=f"e2{m}")
                    nc.scalar.activation(out=e2[:], in_=mean[:], func=AF.Identity,
                                         bias=0.0, scale=2.0)          # 2*mean
                    nc.vector.tensor_scalar(out=e2[:], in0=e2[:], scalar1=cb,
                                            scalar2=cb, op0=OP.subtract, op1=OP.mult)  # (2m-cb)*cb
                    nc.vector.scalar_tensor_tensor(out=e2[:], in0=gstats[:, 2 + m:3 + m],
                                                   scalar=nbl, in1=e2[:],
                                                   op0=OP.mult, op1=OP.add)
                    # var = e2 - mean^2 ; rstd = 1/sqrt(var+eps)
                    nc.vector.tensor_mul(out=tmp1[:], in0=mean[:], in1=mean[:])
                    nc.vector.tensor_tensor(out=e2[:], in0=e2[:], in1=tmp1[:], op=OP.subtract)
                    nc.scalar.activation(out=e2[:], in_=e2[:], func=AF.Ln, bias=eps_t[:])
                    nc.scalar.activation(out=e2[:], in_=e2[:], func=AF.Exp, scale=-0.5)
                    nc.vector.tensor_mul(out=scaleA[:, m:m + 1], in0=e2[:], in1=g)
                    nc.vector.tensor_mul(out=tmp1[:], in0=mean[:], in1=scaleA[:, m:m + 1])
                    nc.vector.tensor_tensor(out=biasA[:, m:m + 1], in0=b, in1=tmp1[:], op=OP.subtract)

                for m in range(2):
                    hsil = front2.tile([128, L], dt.bfloat16, tag="hsil")
                    nc.scalar.activation(out=hsil[:], in_=h_sb[m][:],
                                         func=AF.Silu, bias=biasA[:, m:m + 1],
                                         scale=scaleA[:, m:m + 1])
                    for tt in range(NT):
                        ptp = psum.tile([128, 128], dt.bfloat16, tag="tp", bufs=2)
                        nc.tensor.transpose(ptp[:], hsil[:, tt * 128:(tt + 1) * 128], ident_b[:])
                        nc.scalar.copy(out=res[:, tt * COUT + m * 128: tt * COUT + m * 128 + 128],
                                       in_=ptp[:])

            # ====================== Mamba blocks ======================
            for blk in range(NBLK):
                with tc.tile_pool(name=f"wb{blk}", bufs=1) as wpool, \
                     tc.tile_pool(name=f"blk{blk}", bufs=1) as bpool, \
                     tc.tile_pool(name=f"str{blk}", bufs=2) as spool:
                    # ---- block weights ----
                    winT_sb = [wpool.tile([128, 2 * D_INNER], dt.bfloat16, name=f"winT{kh}") for kh in range(2)]
                    for kh in range(2):
                        nc.sync.dma_start(out=winT_sb[kh][:], in_=winT[blk, kh])
                    winb_sb = wpool.tile([128, 8], dt.float32)
                    nc.sync.dma_start(out=winb_sb[:], in_=winb[blk])
                    cdiag_sb = wpool.tile([128, DG * D_CONV * 128], dt.bfloat16)
                    nc.sync.dma_start(out=cdiag_sb[:], in_=cdiag[blk])
                    cdb_sb = wpool.tile([128, DG], dt.float32)
                    nc.sync.dma_start(out=cdb_sb[:], in_=cdb[blk])
                    xprojT_sb = wpool.tile([128, DG * 48], dt.bfloat16)
                    nc.sync.dma_start(out=xprojT_sb[:], in_=xprojT[blk])
                    dtprojT_sb = wpool.tile([16, D_INNER], dt.bfloat16)
                    nc.sync.dma_start(out=dtprojT_sb[:], in_=dtprojT[blk])
                    dtpb_sb = wpool.tile([128, DG], dt.float32)
                    nc.sync.dma_start(out=dtpb_sb[:], in_=dtpb[blk])
                    acol_sb = wpool.tile([128, DG * D_STATE], dt.float32)
                    nc.sync.dma_start(out=acol_sb[:], in_=acol[blk])
                    dcol_sb = wpool.tile([128, DG], dt.float32)
                    nc.sync.dma_start(out=dcol_sb[:], in_=dcol[blk])
                    wout_sb = [wpool.tile([128, COUT], dt.bfloat16, name=f"wout{dg}") for dg in range(DG)]
                    for dg in range(DG):
                        nc.sync.dma_start(out=wout_sb[dg][:], in_=woutT[blk, dg])

                    # ---- LN + transpose -> xnT ----
                    xnT = [bpool.tile([128, L], dt.bfloat16, name=f"xnT{kh}") for kh in range(2)]
                    for tt in range(NT):
                        stats6 = spool.tile([128, 6], dt.float32, tag="stats6")
                        mv = spool.tile([128, 2], dt.float32, tag="mv")
                        rcol = res[:, tt * COUT:(tt + 1) * COUT]
                        nc.vector.bn_stats(out=stats6[:], in_=rcol)
                        nc.vector.bn_aggr(out=mv[:], in_=stats6[:])
                        rstd = spool.tile([128, 1], dt.float32, tag="rstd")
                        nc.scalar.activation(out=rstd[:], in_=mv[:, 1:2], func=AF.Ln, bias=eps_t[:])
                        nc.scalar.activation(out=rstd[:], in_=rstd[:], func=AF.Exp, scale=-0.5)
                        xn_t = spool.tile([128, COUT], dt.bfloat16, tag="xn_t")
                        nc.vector.tensor_scalar(out=xn_t[:], in0=rcol,
                                                scalar1=mv[:, 0:1], scalar2=rstd[:],
                                                op0=OP.subtract, op1=OP.mult)
                        for kh in range(2):
                            ptp = psum.tile([128, 128], dt.bfloat16, tag="tp", bufs=2)
                            nc.tensor.transpose(ptp[:], xn_t[:, kh * 128:(kh + 1) * 128], ident_b[:])
                            nc.scalar.copy(out=xnT[kh][:, tt * 128:(tt + 1) * 128], in_=ptp[:])

                    # ---- u_raw buffers (3-left-pad), hstate ----
                    uraw = [bpool.tile([128, L + 3], dt.bfloat16, name=f"uraw{dg}") for dg in range(DG)]
                    for dg in range(DG):
                        nc.vector.memset(uraw[dg][:, 0:3], 0.0)
                    hstate = [bpool.tile([128, D_STATE], dt.bfloat16, name=f"hst{dg}") for dg in range(DG)]
                    for dg in range(DG):
                        nc.vector.memset(hstate[dg][:], 0.0)

                    for c in range(NCHUNK):
                        t0 = c * T
                        # ---- in_proj ----
                        zs_c = [spool.tile([128, T], dt.bfloat16, name=f"zs{dg}") for dg in range(DG)]
                        for m in range(8):
                            pxz = psum.tile([128, T], dt.float32, tag="mm512", bufs=3)
                            for kh in range(2):
                                nc.tensor.matmul(
                                    pxz[:], winT_sb[kh][:, m * 128:(m + 1) * 128],
                                    xnT[kh][:, t0:t0 + T],
                                    start=(kh == 0), stop=(kh == 1))
                            if m < 4:
                                nc.scalar.activation(
                                    out=uraw[m][:, 3 + t0:3 + t0 + T], in_=pxz[:],
                                    func=AF.Identity, bias=winb_sb[:, m:m + 1])
                            else:
                                nc.scalar.activation(
                                    out=zs_c[m - 4][:], in_=pxz[:],
                                    func=AF.Silu, bias=winb_sb[:, m:m + 1])
                        # ---- depthwise conv + silu -> u_c ----
                        u_c = [spool.tile([128, T], dt.bfloat16, name=f"u{dg}") for dg in range(DG)]
                        for dg in range(DG):
                            puc = psum.tile([128, T], dt.float32, tag="mm512", bufs=3)
                            for j in range(D_CONV):
                                nc.tensor.matmul(
                                    puc[:],
                                    cdiag_sb[:, (dg * D_CONV + j) * 128:(dg * D_CONV + j + 1) * 128],
                                    uraw[dg][:, t0 + j:t0 + j + T],
                                    start=(j == 0), stop=(j == D_CONV - 1))
                            nc.scalar.activation(out=u_c[dg][:], in_=puc[:],
                                                 func=AF.Silu, bias=cdb_sb[:, dg:dg + 1])
                        # ---- xproj -> x_dbl ----
                        pxd = psum.tile([48, T], dt.float32, tag="mm48", bufs=1)
                        for dg in range(DG):
                            nc.tensor.matmul(pxd[:], xprojT_sb[0:128, dg * 48:(dg + 1) * 48],
                                             u_c[dg][:], start=(dg == 0), stop=(dg == DG - 1))
                        # xproj rows are host-permuted to [B(16), C(16), dt(16)]
                        xd_bc = spool.tile([32, T], dt.bfloat16, tag="xd_bc")
                        nc.vector.tensor_copy(out=xd_bc[:], in_=pxd[0:32, :])
                        xd_dt = spool.tile([16, T], dt.bfloat16, tag="xd_dt")
                        nc.vector.tensor_copy(out=xd_dt[:], in_=pxd[32:48, :])
                        xd_dram = dram.tile([32, T], dt.bfloat16, tag="xd_dram", bufs=2)
                        nc.sync.dma_start(out=xd_dram[:], in_=xd_bc[:])
                        # ---- dtproj + softplus -> dt_c ; dtu ----
                        dt_c = [spool.tile([128, T], dt.bfloat16, name=f"dt{dg}") for dg in range(DG)]
                        dtu_c = [spool.tile([128, T], dt.bfloat16, name=f"dtu{dg}") for dg in range(DG)]
                        for dg in range(DG):
                            pdt = psum.tile([128, T], dt.float32, tag="mm512", bufs=3)
                            nc.tensor.matmul(pdt[:], dtprojT_sb[:, dg * 128:(dg + 1) * 128],
                                             xd_dt[:], start=True, stop=True)
                            # softplus(p) = ln(1 + exp(p)) -- stays in the ln/exp table set
                            e_p = spool.tile([128, T], dt.float32, tag="e_p")
                            nc.scalar.activation(out=e_p[:], in_=pdt[:],
                                                 func=AF.Exp, bias=dtpb_sb[:, dg:dg + 1])
                            nc.scalar.activation(out=dt_c[dg][:], in_=e_p[:],
                                                 func=AF.Ln, bias=1.0)
                            nc.vector.tensor_mul(out=dtu_c[dg][:], in0=dt_c[dg][:], in1=u_c[dg][:])
                        # ---- y_acc init = D*u ----
                        y_acc = [spool.tile([128, T], dt.float32, name=f"yac{dg}") for dg in range(DG)]
                        for dg in range(DG):
                            nc.vector.tensor_scalar_mul(out=y_acc[dg][:], in0=u_c[dg][:],
                                                        scalar1=dcol_sb[:, dg:dg + 1])
                        # ---- scan over s-halves ----
                        for sh in range(SH):
                            brep = spool.tile([128, SHS * T], dt.bfloat16, tag="brep", bufs=1)
                            crep = spool.tile([128, SHS * T], dt.bfloat16, tag="crep", bufs=1)
                            for s8 in range(SHS):
                                s = sh * SHS + s8
                                nc.sync.dma_start(
                                    out=brep[:, s8 * T:(s8 + 1) * T],
                                    in_=xd_dram[s:s + 1, :].to_broadcast([128, T]))
                                nc.sync.dma_start(
                                    out=crep[:, s8 * T:(s8 + 1) * T],
                                    in_=xd_dram[D_STATE + s:D_STATE + s + 1, :].to_broadcast([128, T]))
                            for dg in range(DG):
                                a_sl = spool.tile([128, SHS * T], dt.bfloat16, tag="a_sl", bufs=1)
                                b_sl = spool.tile([128, SHS * T], dt.bfloat16, tag="b_sl", bufs=1)
                                h_sl = spool.tile([128, SHS * T], dt.bfloat16, tag="h_sl", bufs=1)
                                for s8 in range(SHS):
                                    s = sh * SHS + s8
                                    sl = slice(s8 * T, (s8 + 1) * T)
                                    nc.scalar.activation(
                                        out=a_sl[:, sl], in_=dt_c[dg][:], func=AF.Exp,
                                        scale=acol_sb[:, dg * D_STATE + s:dg * D_STATE + s + 1])
                                    nc.vector.tensor_mul(out=b_sl[:, sl], in0=dtu_c[dg][:],
                                                         in1=brep[:, sl])
                                    nc.vector.tensor_tensor_scan(
                                        h_sl[:, sl], a_sl[:, sl], b_sl[:, sl],
                                        hstate[dg][:, s:s + 1],
                                        OP.mult, OP.add)
                                # save chunk-final states (columns T-1, stride T)
                                nc.vector.tensor_copy(
                                    out=hstate[dg][:, sh * SHS:(sh + 1) * SHS],
                                    in_=h_sl[:, T - 1:SHS * T:T])
                                for s8 in range(SHS):
                                    sl = slice(s8 * T, (s8 + 1) * T)
                                    hc = spool.tile([128, T], dt.bfloat16, tag="hc")
                                    nc.vector.tensor_mul(out=hc[:], in0=h_sl[:, sl], in1=crep[:, sl])
                                    nc.vector.tensor_add(out=y_acc[dg][:], in0=y_acc[dg][:], in1=hc[:])
                        # ---- gate + outproj + residual add ----
                        y_g = [spool.tile([128, T], dt.bfloat16, name=f"yg{dg}") for dg in range(DG)]
                        for dg in range(DG):
                            nc.vector.tensor_mul(out=y_g[dg][:], in0=y_acc[dg][:], in1=zs_c[dg][:])
                        for j in range(T // 128):
                            tt = c * (T // 128) + j
                            po = psum.tile([128, COUT], dt.float32, tag="mm256", bufs=2)
                            for dg in range(DG):
                                nc.tensor.matmul(po[:], y_g[dg][:, j * 128:(j + 1) * 128],
                                                 wout_sb[dg][:], start=(dg == 0), stop=(dg == DG - 1))
                            nc.vector.tensor_add(out=res[:, tt * COUT:(tt + 1) * COUT],
                                                 in0=po[:], in1=res[:, tt * COUT:(tt + 1) * COUT])

            # ====================== final LN + outputs ======================
            with tc.tile_pool(name="fin", bufs=1) as fin, \
                 tc.tile_pool(name="fin2", bufs=2) as fin2:
                outT = [fin.tile([128, L], dt.float32, name=f"outT{m}") for m in range(2)]
                for tt in range(NT):
                    rcol = res[:, tt * COUT:(tt + 1) * COUT]
                    stats6 = fin2.tile([128, 6], dt.float32, tag="stats6")
                    mv = fin2.tile([128, 2], dt.float32, tag="mv")
                    nc.vector.bn_stats(out=stats6[:], in_=rcol)
                    nc.vector.bn_aggr(out=mv[:], in_=stats6[:])
                    rstd = fin2.tile([128, 1], dt.float32, tag="rstd")
                    nc.scalar.activation(out=rstd[:], in_=mv[:, 1:2], func=AF.Ln, bias=eps_t[:])
                    nc.scalar.activation(out=rstd[:], in_=rstd[:], func=AF.Exp, scale=-0.5)
                    o_t = fin2.tile([128, COUT], dt.float32, tag="o_t")
                    nc.vector.tensor_scalar(out=o_t[:], in0=rcol,
                                            scalar1=mv[:, 0:1], scalar2=rstd[:],
                                            op0=OP.subtract, op1=OP.mult)
                    nc.vector.tensor_mul(out=o_t[:], in0=o_t[:], in1=lnfw_rep[:])
                    nc.vector.tensor_add(out=o_t[:], in0=o_t[:], in1=lnfb_rep[:])
                    for m in range(2):
                        ptp = psum.tile([128, 128], dt.float32, tag="tp", bufs=2)
                        nc.tensor.transpose(ptp[:], o_t[:, m * 128:(m + 1) * 128], ident_f[:])
                        nc.scalar.copy(out=outT[m][:, tt * 128:(tt + 1) * 128], in_=ptp[:])
                    # residual output
                    nc.sync.dma_start(out=res_out[tt * 128:(tt + 1) * 128, :], in_=rcol)
                for m in range(2):
                    nc.sync.dma_start(out=out_c[m * 128:(m + 1) * 128, :], in_=outT[m][:])

    return nc


# ------------------------------------------------------------- host wrapper
def prep_weights(inputs):
    """Build the weight-layout arrays shared by all cores."""
    f32 = np.float32
    w = {}
    conv_w = np.asarray(inputs["conv_w"], f32)
    w["convw"] = np.ascontiguousarray(conv_w.transpose(2, 1, 0)).astype(BF16)  # (K, CIN, COUT)
    bn = np.zeros((2, 128, 3), f32)
    for m in range(2):
        sl = slice(m * 128, (m + 1) * 128)
        bn[m, :, 0] = np.asarray(inputs["conv_b"], f32)[sl]
        bn[m, :, 1] = np.asarray(inputs["bn_g"], f32)[sl]
        bn[m, :, 2] = np.asarray(inputs["bn_b"], f32)[sl]
    w["bnpar"] = bn
    w["ident_bf"] = np.eye(128, dtype=BF16)
    w["ident_f32"] = np.eye(128, dtype=f32)
    w["lnf"] = np.stack([np.asarray(inputs["lnf_w"], f32), np.asarray(inputs["lnf_b"], f32)])

    winT = np.zeros((NBLK, 2, 128, 2 * D_INNER), BF16)
    winb = np.zeros((NBLK, 128, 8), f32)
    cdiag = np.zeros((NBLK, 128, DG * D_CONV * 128), BF16)
    cdb = np.zeros((NBLK, 128, DG), f32)
    xprojT = np.zeros((NBLK, 128, DG * 48), BF16)
    dtprojT = np.zeros((NBLK, 16, D_INNER), BF16)
    dtpb = np.zeros((NBLK, 128, DG), f32)
    acol = np.zeros((NBLK, 128, DG * D_STATE), f32)
    dcol = np.zeros((NBLK, 128, DG), f32)
    woutT = np.zeros((NBLK, DG, 128, COUT), BF16)
    for i in range(NBLK):
        W_eff = (np.asarray(inputs["in_proj_w"][i], f32) *
                 np.asarray(inputs["blk_ln_w"][i], f32)[None, :])          # (1024, 256)
        b_eff = np.asarray(inputs["in_proj_w"][i], f32) @ np.asarray(inputs["blk_ln_b"][i], f32)
        WT = W_eff.T                                                        # (256, 1024)
        winT[i, 0] = WT[:128].astype(BF16)
        winT[i, 1] = WT[128:].astype(BF16)
        winb[i] = b_eff.reshape(8, 128).T
        cw = np.asarray(inputs["convd_w"][i][:, 0, :], f32)                 # (512, 4)
        for dg in range(DG):
            for j in range(D_CONV):
                cdiag[i, :, (dg * D_CONV + j) * 128:(dg * D_CONV + j + 1) * 128] = \
                    np.diag(cw[dg * 128:(dg + 1) * 128, j]).astype(BF16)
        cdb[i] = np.asarray(inputs["convd_b"][i], f32).reshape(DG, 128).T
        xp_perm = np.concatenate([
            np.asarray(inputs["xproj_w"][i], f32)[DT_RANK:],      # B(16), C(16) first
            np.asarray(inputs["xproj_w"][i], f32)[:DT_RANK],      # dt last
        ], axis=0)
        xp = xp_perm.T                                                      # (512, 48)
        for dg in range(DG):
            xprojT[i, :, dg * 48:(dg + 1) * 48] = xp[dg * 128:(dg + 1) * 128].astype(BF16)
        dtprojT[i] = np.asarray(inputs["dtproj_w"][i], f32).T.astype(BF16)  # (16, 512)
        dtpb[i] = np.asarray(inputs["dtproj_b"][i], f32).reshape(DG, 128).T
        A = -np.exp(np.asarray(inputs["A_log"][i], f32))                    # (512, 16)
        for dg in range(DG):
            acol[i, :, dg * D_STATE:(dg + 1) * D_STATE] = A[dg * 128:(dg + 1) * 128]
        dcol[i] = np.asarray(inputs["Dp"][i], f32).reshape(DG, 128).T
        WoT = np.asarray(inputs["outproj_w"][i], f32).T                     # (512, 256)
        for dg in range(DG):
            woutT[i, dg] = WoT[dg * 128:(dg + 1) * 128].astype(BF16)
    w.update(winT=winT, winb=winb, cdiag=cdiag, cdb=cdb, xprojT=xprojT,
             dtprojT=dtprojT, dtpb=dtpb, acol=acol, dcol=dcol, woutT=woutT)
    return w


_RUNNER = None


def _build_runner():
    """Build nc + a cached jitted SPMD callable (mirrors run_bass_via_pjrt)."""
    _install_wait_legalizer()
    import jax
    import jax.numpy as jnp
    from jax.experimental.shard_map import shard_map
    from jax.sharding import Mesh, PartitionSpec
    import concourse.bass2jax as bass2jax
    import concourse.mybir as mybir

    bass2jax.install_neuronx_cc_hook()
    nc = build_program(N_CORES)

    in_names, out_names, out_avals, zero_outs = [], [], [], []
    partition_name = nc.partition_id_tensor.name if nc.partition_id_tensor else None
    for alloc in nc.m.functions[0].allocations:
        if not isinstance(alloc, mybir.MemoryLocationSet):
            continue
        name = alloc.memorylocations[0].name
        if alloc.kind == "ExternalInput":
            if name != partition_name:
                in_names.append(name)
        elif alloc.kind == "ExternalOutput":
            shape = tuple(alloc.tensor_shape)
            dtype = mybir.dt.np(alloc.dtype)
            out_names.append(name)
            out_avals.append(jax.core.ShapedArray(shape, dtype))
            zero_outs.append(np.zeros(shape, dtype))
    n_params = len(in_names)
    n_outs = len(out_names)
    all_in_names = list(in_names) + list(out_names)
    if partition_name is not None:
        all_in_names.append(partition_name)
    donate = tuple(range(n_params, n_params + n_outs))

    def _body(*args):
        operands = list(args)
        if partition_name is not None:
            operands.append(bass2jax.partition_id_tensor())
        outs = bass2jax._bass_exec_p.bind(
            *operands,
            out_avals=tuple(out_avals),
            in_names=tuple(all_in_names),
            out_names=tuple(out_names),
            lowering_input_output_aliases=(),
            sim_require_finite=True,
            sim_require_nnan=True,
            nc=nc,
        )
        return tuple(outs)

    devices = jax.devices()[:N_CORES]
    mesh = Mesh(np.asarray(devices), ("core",))
    in_specs = (PartitionSpec("core"),) * (n_params + n_outs)
    out_specs = (PartitionSpec("core"),) * n_outs
    sharded = jax.jit(
        shard_map(_body, mesh=mesh, in_specs=in_specs, out_specs=out_specs,
                  check_rep=False),
        donate_argnums=donate, keep_unused=True)

    def run(in_maps):
        concat_in = [
            np.concatenate([np.asarray(in_maps[c][name]) for c in range(N_CORES)], axis=0)
            for name in in_names
        ]
        concat_zeros = [np.zeros((N_CORES * z.shape[0], *z.shape[1:]), z.dtype)
                        for z in zero_outs]
        out_arrs = sharded(*concat_in, *concat_zeros)
        return [
            {name: np.asarray(out_arrs[i]).reshape(N_CORES, *out_avals[i].shape)[c]
             for i, name in enumerate(out_names)}
            for c in range(N_CORES)
        ]

    return run


def get_runner():
    global _RUNNER
    if _RUNNER is None:
        _RUNNER = _build_runner()
    return _RUNNER


def make_in_maps(inputs):
    w = prep_weights(inputs)
    x = np.asarray(inputs["x"], np.float32)
    in_maps = []
    for c in range(N_CORES):
        m = dict(w)
        m["x"] = np.ascontiguousarray(x[c])
        in_maps.append(m)
    return in_maps


def kernel(**inputs):
    run = get_runner()
    results = run(make_in_maps(inputs))
    out = np.stack([results[c]["out_c"] for c in range(N_CORES)])      # (8, 256, 4096)
    res = np.stack([results[c]["res_out"] for c in range(N_CORES)])    # (8, 4096, 256)
    return out, res
